# revision 22
# baseline (speedup 1.0000x reference)
"""Trainium2 Bass kernel for nn_Block_69861938036758 (sparse_attention).

Data-parallel over batch B=8 across 8 NeuronCores (one batch element per
core, no collectives). Per core, one fused transformer block:

  LN1 -> per-modality QKV -> masked per-modality softmax with
  modality-importance weighting -> AV -> proj -> residual -> LN2 ->
  fc1 -> exact gelu -> fc2 -> residual

Layout choices (matmul operands bf16, fp32 accumulation everywhere):
  - Scores computed transposed: S^T[ktok, qtok], so the softmax key-sums are
    matmul contractions and the prob matrix is never transposed.
  - Per-modality row-sums ride free as an extra ones-column in V.
  - Modality-importance means via the factorization
      mean[m,h] = SCALE/(nz_m*N) * <qsum_h, ksum_{m,h}>.
  - The modality weight (softmaxed importance) is folded into the PSUM->SBUF
    evacuation of U as a per-partition tensor_scalar multiply; per-query
    normalization is then a single bf16 tensor_tensor DIVIDE against the
    broadcast row-sums (no reciprocal, no fp32 combine chain).
  - Weight DMAs ordered x -> wq -> wk -> wv; pw/f1/f2 prefetched on the sync
    queue during attention so the MLP never waits on HBM.
"""
import numpy as np
import ml_dtypes
from contextlib import ExitStack

import concourse.bass as bass
import concourse.bacc as bacc
import concourse.tile as tile
from concourse import mybir
from concourse.bass_utils import run_bass_kernel_spmd
from concourse.masks import make_identity

F32 = mybir.dt.float32
BF16 = mybir.dt.bfloat16
FP8 = mybir.dt.float8e4
DR = mybir.MatmulPerfMode.DoubleRow
AF = mybir.ActivationFunctionType
ALU = mybir.AluOpType

SIZES = (256, 512, 256)
H, DIM, HD, N = 12, 768, 64, 1024
NT = N // 128            # 8 token tiles
SCALE = HD ** -0.5
MB = -30000.0            # additive mask bias (exp underflows to exact 0)
MOD_OFF = (0, 256, 768)
MOD_CHUNKS = ((0, 2), (2, 6), (6, 8))   # ktok 128-chunk ranges per modality
MT_MOD = (0, 0, 1, 1, 1, 1, 2, 2)       # modality of each 128-token tile
EPS = 1e-5

_CACHE = {}


def _emit(ctx, tc, nc, T, flags, sfx="", out_name="out"):
    apply_ln1, apply_ln2, use_pb, use_f1b, use_f2b, need_eps = flags
    v = nc.vector
    s = nc.scalar
    pe = nc.tensor

    const = ctx.enter_context(tc.tile_pool(name="const" + sfx, bufs=1))
    drs = ctx.enter_context(tc.tile_pool(name="drs" + sfx, bufs=4, space="DRAM"))

    # x streams first so LN1 can start immediately.
    p_x = tc.alloc_tile_pool(name="p_x" + sfx, bufs=1)          # [A..E]
    x_sb = p_x.tile([128, NT, DIM], F32)
    nc.sync.dma_start(out=x_sb[:], in_=T["x"].ap())

    ident = const.tile([128, 128], BF16)
    make_identity(nc, ident[:])

    # const-table DMAs ride the gpsimd queue so the sync queue streams
    # x -> wq -> wk -> wv back-to-back.
    mb_sb = const.tile([128, 8], F32)
    nc.gpsimd.dma_start(out=mb_sb[:], in_=T["mb2d"].ap())
    maskrep = const.tile([128, N], F32)
    mf = T["maskf"].ap()
    nc.gpsimd.dma_start(out=maskrep[:], in_=bass.AP(
        tensor=mf.tensor, offset=mf.offset, ap=[[0, 128], [1, N]]))
    denr = const.tile([128, 3], F32)
    d3 = T["den3"].ap()
    nc.gpsimd.dma_start(out=denr[:], in_=bass.AP(
        tensor=d3.tensor, offset=d3.offset, ap=[[0, 128], [1, 3]]))
    br12 = const.tile([12, 3], F32)
    b3 = T["bias3"].ap()
    nc.gpsimd.dma_start(out=br12[:], in_=bass.AP(
        tensor=b3.tensor, offset=b3.offset, ap=[[0, 12], [1, 3]]))
    seg_sb = const.tile([128, 6, 12], BF16)
    nc.gpsimd.dma_start(out=seg_sb[:], in_=T["seg"].ap())
    eps_t = const.tile([128, 1], F32)
    v.memset(eps_t[:], EPS)
    # mscale[p, h, m]: modality weight for partitions 0..63, 1.0 at 64..127
    # (so the ones-column row-sum row of U rides through evacuation unscaled).
    mscale = const.tile([128, 12, 3], F32)
    v.memset(mscale[:], 1.0)

    def brep(name):
        t = const.tile([128, DIM], F32, tag=name)
        ap = T[name].ap()
        nc.sync.dma_start(out=t[:], in_=bass.AP(
            tensor=ap.tensor, offset=ap.offset, ap=[[0, 128], [1, DIM]]))
        return t

    ln1w_r = brep("ln1w") if apply_ln1 else None
    ln1b_r = brep("ln1b") if apply_ln1 else None
    ln2w_r = brep("ln2w") if apply_ln2 else None
    ln2b_r = brep("ln2b") if apply_ln2 else None
    pb_r = brep("pb") if use_pb else None
    f2b_r = brep("f2b") if use_f2b else None
    f1b_sb = None
    if use_f1b:
        f1b_sb = const.tile([128, 24], F32)
        nc.sync.dma_start(out=f1b_sb[:], in_=T["f1b"].ap())

    # manually-managed pools; stack (LIFO) allocator => nest lifetimes,
    # with weight-prefetch pools on the right-side stack.
    p_ot = tc.alloc_tile_pool(name="p_ot" + sfx, bufs=1)        # [A..E]
    p_qktv = tc.alloc_tile_pool(name="p_qktv" + sfx, bufs=1)    # [A..D]
    OT = p_ot.tile([128, 6, N], BF16)
    QT = p_qktv.tile([128, 6, N], BF16)
    KT = p_qktv.tile([128, 6, N], BF16)
    V = p_qktv.tile([128, NT, H, HD + 1], BF16)

    v.memset(V[:, :, :, HD:HD + 1], 1.0)  # ones column -> free row-sums

    # modality-importance statistics accumulate during phase B; released
    # after phase C (stacked above p_qktv, LIFO).
    stat = tc.alloc_tile_pool(name="stat" + sfx, bufs=1)
    statw = tc.alloc_tile_pool(name="statw" + sfx, bufs=2)
    qs = stat.tile([128, 6], F32)
    ks = stat.tile([128, 6, 3], F32)
    prod = stat.tile([128, 6, 3], BF16)

    def layer_norm_into(pool, src_ap, wr, br_, tag):
        """LN over free dim (768) -> bf16 [128, 768] tile."""
        stats = pool.tile([128, 3, 6], F32, tag=tag + "_st")
        for i in range(3):
            v.bn_stats(out=stats[:, i, :], in_=src_ap[:, i * 256:(i + 1) * 256])
        mv = pool.tile([128, 2], F32, tag=tag + "_mv")
        v.bn_aggr(out=mv[:], in_=stats[:])
        sd = pool.tile([128, 1], F32, tag=tag + "_sd")
        s.activation(sd[:], mv[:, 1:2], AF.Sqrt, bias=eps_t[:])
        rstd = pool.tile([128, 1], F32, tag=tag + "_rs")
        v.reciprocal(rstd[:], sd[:])
        out_bf = pool.tile([128, DIM], BF16, tag=tag + "_o")
        if wr is None:
            v.tensor_scalar(out_bf[:], src_ap, mv[:, 0:1], rstd[:],
                            op0=ALU.subtract, op1=ALU.mult)
        else:
            tmp = pool.tile([128, DIM], F32, tag=tag + "_t")
            v.tensor_scalar(tmp[:], src_ap, mv[:, 0:1], rstd[:],
                            op0=ALU.subtract, op1=ALU.mult)
            v.tensor_tensor(tmp[:], tmp[:], wr[:], op=ALU.mult)
            v.tensor_tensor(out_bf[:], tmp[:], br_[:], op=ALU.add)
        return out_bf

    # ---- Phase A: LN1 + transpose to xnT; Phase B: QKV ----------------------
    with tc.tile_pool(name="qkvw" + sfx, bufs=1) as qkvw:

        wq_sb = qkvw.tile([128, 3, 6, 6, 128], FP8)
        wk_sb = qkvw.tile([128, 3, 6, 6, 128], FP8)
        wv_sb = qkvw.tile([128, 3, 6, DIM], FP8)
        for m in range(3):
            nc.sync.dma_start(out=wq_sb[:, m, :, :, :], in_=T["wq"].ap()[:, m])
        for m in range(3):
            nc.sync.dma_start(out=wk_sb[:, m, :, :, :], in_=T["wk"].ap()[:, m])
        for m in range(3):
            nc.sync.dma_start(out=wv_sb[:, m, :, :], in_=T["wv"].ap()[:, m])
        xnT = qkvw.tile([128, 6, N], FP8)

        with tc.tile_pool(name="lnp" + sfx, bufs=3) as lnp, \
             tc.tile_pool(name="ptp" + sfx, bufs=3, space="PSUM") as ptp:
            for mt in range(NT):
                xn = layer_norm_into(lnp, x_sb[:, mt, :], ln1w_r, ln1b_r, "ln1")
                for kc in range(6):
                    pt = ptp.tile([128, 128], BF16, tag="tp")
                    pe.transpose(pt[:], xn[:, kc * 128:(kc + 1) * 128], ident[:])
                    s.copy(xnT[:, kc, mt * 128:(mt + 1) * 128], pt[:])

        with tc.tile_pool(name="pqv" + sfx, bufs=2, space="PSUM") as pqv:
            for w_sb, dst, is_q in ((wq_sb, QT, True), (wk_sb, KT, False)):
                for pc in range(6):
                    ps = pqv.tile([128, N], F32, tag="q")
                    for seg, m in ((0, 0), (1, 1), (2, 1), (3, 2)):
                        o = seg * 256
                        for kc in range(0, 6, 2):
                            pe.matmul(ps[:, o:o + 256],
                                      w_sb[:, m, kc:kc + 2, pc, :],
                                      xnT[:, kc:kc + 2, o:o + 256],
                                      start=(kc == 0), stop=(kc == 4),
                                      perf_mode=DR)
                    s.copy(dst[:, pc, :], ps[:])
                    # importance stats ride along on otherwise-idle DVE
                    if is_q:
                        v.reduce_sum(qs[:, pc:pc + 1], dst[:, pc, :],
                                     axis=mybir.AxisListType.X)
                    else:
                        km = statw.tile([128, N], F32, tag="km")
                        v.tensor_tensor(km[:], dst[:, pc, :], maskrep[:],
                                        op=ALU.mult)
                        for m in range(3):
                            o, sz = MOD_OFF[m], SIZES[m]
                            v.reduce_sum(ks[:, pc, m:m + 1], km[:, o:o + sz],
                                         axis=mybir.AxisListType.X)
            for mt in range(NT):
                m = MT_MOD[mt]
                ps = pqv.tile([128, DIM], F32, tag="v")
                for fo, fs in ((0, 512), (512, 256)):
                    for kc in range(0, 6, 2):
                        pe.matmul(ps[:, fo:fo + fs],
                                  xnT[:, kc:kc + 2, mt * 128:(mt + 1) * 128],
                                  wv_sb[:, m, kc:kc + 2, fo:fo + fs],
                                  start=(kc == 0), stop=(kc == 4),
                                  perf_mode=DR)
                # V evacuation on DVE keeps ACT clear for the first exps
                v.tensor_copy(V[:, mt, :, 0:HD],
                              ps[:].rearrange("p (h d) -> p h d", h=H))
                if mt == 3:
                    # ---- Phase C: modality-importance means -> mscale ------
                    # Spliced mid-V-loop: the DVE stats are ready by now, the
                    # pm matmul slots between V matmuls with no PE stall, and
                    # the scm->mscale DMA round-trip (~3us SWDGE completion
                    # latency) finishes under the rest of the V-loop instead
                    # of stalling the first exp/evacuation of phase D.
                    for m_ in range(3):
                        v.tensor_scalar(ks[:, :, m_], ks[:, :, m_],
                                        denr[:, m_:m_ + 1], SCALE,
                                        op0=ALU.mult, op1=ALU.mult)
                    for pc in range(6):
                        v.tensor_scalar(prod[:, pc, :], ks[:, pc, :],
                                        qs[:, pc:pc + 1], None, op0=ALU.mult)
                    pm = pqv.tile([12, 3], F32, tag="v")  # borrow a v-slot
                    for pc in range(6):
                        pe.matmul(pm[:], seg_sb[:, pc, :], prod[:, pc, :],
                                  start=(pc == 0), stop=(pc == 5))
                    mn = stat.tile([12, 3], F32)
                    v.tensor_tensor(mn[:], pm[:], br12[:], op=ALU.add)
                    me = stat.tile([12, 3], F32)
                    msum = stat.tile([12, 1], F32)
                    s.activation(me[:], mn[:], AF.Exp, accum_out=msum[:])
                    mrec = stat.tile([12, 1], F32)
                    v.reciprocal(mrec[:], msum[:])
                    mult_sb = stat.tile([12, 3], F32)
                    v.tensor_scalar(mult_sb[:], me[:], mrec[:], None,
                                    op0=ALU.mult)
                    scm = drs.tile([12, 3], F32, tag="scm")
                    nc.gpsimd.dma_start(out=scm[:], in_=mult_sb[:])
                    sc = scm[:]
                    nc.gpsimd.dma_start(out=mscale[0:64, :, :], in_=bass.AP(
                        tensor=sc.tensor, offset=sc.offset,
                        ap=[[0, 64], [3, 12], [1, 3]]))

    statw.release()
    stat.release()

    # prefetch proj + MLP weights on the sync queue; they land during
    # attention so proj/fc1/fc2 never wait on HBM.
    p_pw = tc.alloc_tile_pool(name="p_pw" + sfx, bufs=1, side="right")
    pw_sb = p_pw.tile([128, 6, DIM], BF16)
    nc.sync.dma_start(out=pw_sb[:], in_=T["pw"].ap())
    p_w1 = tc.alloc_tile_pool(name="p_w1" + sfx, bufs=1, side="right")
    f1_sb = p_w1.tile([128, 6, 4 * DIM], BF16)
    nc.sync.dma_start(out=f1_sb[:], in_=T["f1"].ap())

    # ---- Phase D: attention (transposed scores) -----------------------------
    # Full-width (1024) score/exp instructions. U accumulators are evacuated
    # from PSUM to SBUF bf16 with the modality weight folded in (per-partition
    # tensor_scalar); per-query normalization is a bf16 tensor_tensor DIVIDE
    # against the DMA-broadcast row-sums.
    with tc.tile_pool(name="pst" + sfx, bufs=2, space="PSUM") as pst, \
         tc.tile_pool(name="pu" + sfx, bufs=2, space="PSUM") as pu, \
         tc.tile_pool(name="ep" + sfx, bufs=4) as ep, \
         tc.tile_pool(name="usb" + sfx, bufs=9) as usb, \
         tc.tile_pool(name="r6p" + sfx, bufs=3) as r6p, \
         tc.tile_pool(name="rp" + sfx, bufs=9) as rp, \
         tc.tile_pool(name="cp" + sfx, bufs=3) as cp, \
         nc.allow_low_precision(reason="bf16 attention combine; tolerance 2e-2"):
        for h in range(H):
            # row-sum rows collect on partitions 0..2 of one tile, so a
            # single reciprocal (free-dim cost only) serves all 3 modalities.
            r6 = r6p.tile([3, N], BF16, tag="r6", name=f"r6_{h}")
            Us = {}
            if True:
                po = (h % 2) * 64
                pc = h // 2
                for m in range(3):
                    c0, c1 = MOD_CHUNKS[m]
                    U = pu.tile([HD + 1, N], F32, tag="u", name=f"u_{h}_{m}")
                    for c in range(c0, c1):
                        st = pst.tile([128, N], F32, tag="st", name=f"st_{h}_{c}")
                        for half in range(2):
                            hs = slice(half * 512, (half + 1) * 512)
                            pe.matmul(st[:, hs],
                                      KT[po:po + 64, pc, c * 128:(c + 1) * 128],
                                      QT[po:po + 64, pc, hs],
                                      start=True, stop=True)
                        E = ep.tile([128, N], BF16, tag="e", name=f"e_{h}_{c}")
                        s.activation(E[:], st[:], AF.Exp,
                                     bias=mb_sb[:, c:c + 1], scale=SCALE)
                        for half in range(2):
                            hs = slice(half * 512, (half + 1) * 512)
                            pe.matmul(U[:, hs], V[:, c, h, :], E[:, hs],
                                      start=(c == c0), stop=(c == c1 - 1))
                    Usb = usb.tile([HD + 1, N], BF16, tag="usb",
                                   name=f"usb_{h}_{m}")
                    # evacuate PSUM with the modality weight folded in; row 64
                    # (the row-sum) is scaled by 1.0.
                    v.tensor_scalar(Usb[:], U[:], mscale[0:HD + 1, h, m:m + 1],
                                    None, op0=ALU.mult)
                    nc.sync.dma_start(out=r6[m:m + 1, :],
                                       in_=Usb[64:65, :])
                    Us[(h, m)] = Usb
            if need_eps:
                v.tensor_scalar(r6[:], r6[:], 1e-12, None, op0=ALU.add)
            v.reciprocal(r6[:], r6[:])
            rr = drs.tile([3, N], BF16, tag="scr", name=f"rr_{h}")
            nc.sync.dma_start(out=rr[:], in_=r6[:])
            if True:
                Rs = []
                for m in range(3):
                    idx = m
                    ra = rr[idx:idx + 1, :]
                    Rm = rp.tile([64, N], BF16, tag="rm", name=f"rm_{h}_{m}")
                    nc.gpsimd.dma_start(out=Rm[:], in_=bass.AP(
                        tensor=ra.tensor, offset=ra.offset,
                        ap=[[0, 64], [1, N]]))
                    Rs.append(Rm)
                acc = cp.tile([64, N], BF16, tag="acc", name=f"acc_{h}")
                v.tensor_tensor(acc[:], Us[(h, 0)][0:64, :], Rs[0][:],
                                op=ALU.mult)
                t1 = cp.tile([64, N], BF16, tag="t1")
                v.tensor_tensor(t1[:], Us[(h, 1)][0:64, :], Rs[1][:],
                                op=ALU.mult)
                v.tensor_tensor(acc[:], acc[:], t1[:], op=ALU.add)
                t2 = cp.tile([64, N], BF16, tag="t2")
                v.tensor_tensor(t2[:], Us[(h, 2)][0:64, :], Rs[2][:],
                                op=ALU.mult)
                v.tensor_tensor(OT[po:po + 64, pc, :], acc[:], t2[:],
                                op=ALU.add)

    p_qktv.release()

    # ---- Phase E+F fused: proj + residual + LN2 + transpose, per tile -------
    # One loop per token tile keeps PE (proj matmuls + transposes), DVE
    # (residual + LN2) and ACT (hT evacuation) pipelined instead of running
    # three serial engine-queues.
    p_x2h = tc.alloc_tile_pool(name="p_x2h" + sfx, bufs=1, side="right")  # [E..H]
    x2_sb = p_x2h.tile([128, NT, DIM], F32)
    hT = p_x2h.tile([128, 6, N], BF16)
    p_w2 = tc.alloc_tile_pool(name="p_w2" + sfx, bufs=1, side="right")
    f2_sb = p_w2.tile([128, 24, DIM], BF16)
    nc.sync.dma_start(out=f2_sb[:], in_=T["f2"].ap())
    with tc.tile_pool(name="py" + sfx, bufs=2, space="PSUM") as py, \
         tc.tile_pool(name="lnp2" + sfx, bufs=3) as lnp2, \
         tc.tile_pool(name="ptp2" + sfx, bufs=2, space="PSUM") as ptp2:
        for mt in range(NT):
            ps = py.tile([128, DIM], F32, tag="y")
            for fo, fs in ((0, 512), (512, 256)):
                for pc in range(6):
                    pe.matmul(ps[:, fo:fo + fs],
                              OT[:, pc, mt * 128:(mt + 1) * 128],
                              pw_sb[:, pc, fo:fo + fs],
                              start=(pc == 0), stop=(pc == 5))
            if use_pb:
                v.tensor_tensor(ps[:], ps[:], pb_r[:], op=ALU.add)
            v.tensor_tensor(x2_sb[:, mt, :], ps[:], x_sb[:, mt, :], op=ALU.add)
            hn = layer_norm_into(lnp2, x2_sb[:, mt, :], ln2w_r, ln2b_r, "ln2")
            for kc in range(6):
                pt = ptp2.tile([128, 128], BF16, tag="tp2")
                pe.transpose(pt[:], hn[:, kc * 128:(kc + 1) * 128], ident[:])
                s.copy(hT[:, kc, mt * 128:(mt + 1) * 128], pt[:])

    p_ot.release()
    p_x.release()

    # ---- Phase G/H: MLP -----------------------------------------------------
    with tc.tile_pool(name="mlpg" + sfx, bufs=1) as mlpg, \
         tc.tile_pool(name="pg" + sfx, bufs=2, space="PSUM") as pg, \
         tc.tile_pool(name="pz" + sfx, bufs=2, space="PSUM") as pz, \
         tc.tile_pool(name="op" + sfx, bufs=3) as op:
        gT = mlpg.tile([128, 24, N], BF16)
        # half-token passes: fc1 on tokens 0..511 starts as soon as the first
        # four LN2 tiles are transposed, overlapping the rest of phase F.
        for half in range(2):
            hs = slice(half * 512, (half + 1) * 512)
            for oc in range(24):
                ps = pg.tile([128, 512], F32, tag="g")
                for kc in range(6):
                    pe.matmul(ps[:], f1_sb[:, kc, oc * 128:(oc + 1) * 128],
                              hT[:, kc, hs], start=(kc == 0), stop=(kc == 5))
                if use_f1b:
                    s.activation(gT[:, oc, hs], ps[:], AF.Gelu,
                                 bias=f1b_sb[:, oc:oc + 1])
                else:
                    s.activation(gT[:, oc, hs], ps[:], AF.Gelu)
        for t in range(NT):
            ps = pz.tile([128, DIM], F32, tag="z")
            for fo, fs in ((0, 512), (512, 256)):
                for oc in range(24):
                    pe.matmul(ps[:, fo:fo + fs],
                              gT[:, oc, t * 128:(t + 1) * 128],
                              f2_sb[:, oc, fo:fo + fs],
                              start=(oc == 0), stop=(oc == 23))
            ob = op.tile([128, DIM], F32, tag="ob")
            if use_f2b:
                v.tensor_tensor(ob[:], ps[:], f2b_r[:], op=ALU.add)
                v.tensor_tensor(ob[:], ob[:], x2_sb[:, t, :], op=ALU.add)
            else:
                v.tensor_tensor(ob[:], ps[:], x2_sb[:, t, :], op=ALU.add)
            nc.gpsimd.dma_start(out=T[out_name].ap()[t], in_=ob[:])
    p_w2.release()
    p_x2h.release()
    p_w1.release()
    p_pw.release()


def _build(flags, reps=1):
    nc = bacc.Bacc("TRN2", target_bir_lowering=False, debug=False, num_devices=8)
    apply_ln1, apply_ln2, use_pb, use_f1b, use_f2b, need_eps = flags
    T = {}
    T["x"] = nc.dram_tensor("x", (128, NT, DIM), F32, kind="ExternalInput")
    T["maskf"] = nc.dram_tensor("maskf", (N,), F32, kind="ExternalInput")
    T["mb2d"] = nc.dram_tensor("mb2d", (128, 8), F32, kind="ExternalInput")
    T["den3"] = nc.dram_tensor("den3", (3,), F32, kind="ExternalInput")
    T["bias3"] = nc.dram_tensor("bias3", (3,), F32, kind="ExternalInput")
    T["seg"] = nc.dram_tensor("seg", (128, 6, 12), BF16, kind="ExternalInput")
    T["wq"] = nc.dram_tensor("wq", (128, 3, 6, 6, 128), FP8, kind="ExternalInput")
    T["wk"] = nc.dram_tensor("wk", (128, 3, 6, 6, 128), FP8, kind="ExternalInput")
    T["wv"] = nc.dram_tensor("wv", (128, 3, 6, DIM), FP8, kind="ExternalInput")
    T["pw"] = nc.dram_tensor("pw", (128, 6, DIM), BF16, kind="ExternalInput")
    T["f1"] = nc.dram_tensor("f1", (128, 6, 4 * DIM), BF16, kind="ExternalInput")
    T["f2"] = nc.dram_tensor("f2", (128, 24, DIM), BF16, kind="ExternalInput")
    if apply_ln1:
        T["ln1w"] = nc.dram_tensor("ln1w", (DIM,), F32, kind="ExternalInput")
        T["ln1b"] = nc.dram_tensor("ln1b", (DIM,), F32, kind="ExternalInput")
    if apply_ln2:
        T["ln2w"] = nc.dram_tensor("ln2w", (DIM,), F32, kind="ExternalInput")
        T["ln2b"] = nc.dram_tensor("ln2b", (DIM,), F32, kind="ExternalInput")
    if use_pb:
        T["pb"] = nc.dram_tensor("pb", (DIM,), F32, kind="ExternalInput")
    if use_f1b:
        T["f1b"] = nc.dram_tensor("f1b", (128, 24), F32, kind="ExternalInput")
    if use_f2b:
        T["f2b"] = nc.dram_tensor("f2b", (DIM,), F32, kind="ExternalInput")
    for r in range(reps):
        T[f"out{r}"] = nc.dram_tensor(f"out{r}", (NT, 128, DIM), F32,
                                      kind="ExternalOutput")

    with tile.TileContext(nc) as tc:
        for r in range(reps):
            with ExitStack() as ctx:
                _emit(ctx, tc, nc, T, flags, sfx=f"_{r}", out_name=f"out{r}")
    nc.compile()
    return nc


def get_program(flags, reps=1):
    key = (flags, reps)
    if key not in _CACHE:
        _CACHE[key] = _build(flags, reps)
    return _CACHE[key]


def _bf(a):
    return np.ascontiguousarray(a, dtype=np.float32).astype(ml_dtypes.bfloat16)


def _f8(a):
    a = np.clip(np.ascontiguousarray(a, dtype=np.float32), -240.0, 240.0)
    return a.astype(ml_dtypes.float8_e4m3)


def prepare(inputs):
    """Host-side prep: flags + per-core input maps."""
    x = np.asarray(inputs["x"], np.float32)
    mask = np.asarray(inputs["attention_mask"])
    ln1_w = np.asarray(inputs["ln1_w"], np.float32)
    ln1_b = np.asarray(inputs["ln1_b"], np.float32)
    ln2_w = np.asarray(inputs["ln2_w"], np.float32)
    ln2_b = np.asarray(inputs["ln2_b"], np.float32)
    proj_b = np.asarray(inputs["proj_b"], np.float32)
    fc1_b = np.asarray(inputs["fc1_b"], np.float32)
    fc2_b = np.asarray(inputs["fc2_b"], np.float32)
    qkv_ws = [np.asarray(inputs[k], np.float32)
              for k in ("qkv_text_w", "qkv_video_w", "qkv_audio_w")]
    proj_w = np.asarray(inputs["proj_w"], np.float32)
    fc1_w = np.asarray(inputs["fc1_w"], np.float32)
    fc2_w = np.asarray(inputs["fc2_w"], np.float32)

    nz_all = []
    for b in range(x.shape[0]):
        mf = (mask[b] != 0)
        nz_all.append([mf[o:o + sz].sum() for o, sz in zip(MOD_OFF, SIZES)])
    flags = (
        not (np.all(ln1_w == 1.0) and np.all(ln1_b == 0.0)),
        not (np.all(ln2_w == 1.0) and np.all(ln2_b == 0.0)),
        bool(np.any(proj_b != 0.0)),
        bool(np.any(fc1_b != 0.0)),
        bool(np.any(fc2_b != 0.0)),
        bool(np.any(np.array(nz_all) == 0)),
    )
    apply_ln1, apply_ln2, use_pb, use_f1b, use_f2b, need_eps = flags

    # shared (identical per core) tensors, DMA-friendly partition-first layouts
    def pack_qk(rows):
        # (3 modalities, 768 out, 768 in) -> [128 p(featin), 3, 6 kc, 6 pc, 128]
        a = np.stack([w[rows[0]:rows[1]].T.reshape(6, 128, 6, 128)
                      for w in qkv_ws])            # (3, kc, p, pc, j)
        return _f8(a.transpose(2, 0, 1, 3, 4))

    wq = pack_qk((0, 768))
    wk = pack_qk((768, 1536))
    wv = _f8(np.stack([w[1536:2304].T.reshape(6, 128, DIM) for w in qkv_ws])
             .transpose(2, 0, 1, 3))               # [128, 3, 6, 768]
    pw = _bf(proj_w.T.reshape(6, 128, DIM).transpose(1, 0, 2))
    f1 = _bf(fc1_w.T.reshape(6, 128, 4 * DIM).transpose(1, 0, 2))
    f2 = _bf(fc2_w.T.reshape(24, 128, DIM).transpose(1, 0, 2))
    seg = np.zeros((128, 6, 12), np.float32)
    for pc in range(6):
        seg[0:64, pc, 2 * pc] = 1.0
        seg[64:128, pc, 2 * pc + 1] = 1.0
    shared = {"wq": wq, "wk": wk, "wv": wv, "pw": pw, "f1": f1, "f2": f2,
              "seg": _bf(seg)}
    if apply_ln1:
        shared["ln1w"], shared["ln1b"] = ln1_w, ln1_b
    if apply_ln2:
        shared["ln2w"], shared["ln2b"] = ln2_w, ln2_b
    if use_pb:
        shared["pb"] = proj_b
    if use_f1b:
        shared["f1b"] = np.ascontiguousarray(fc1_b.reshape(24, 128).T)
    if use_f2b:
        shared["f2b"] = fc2_b

    in_maps = []
    for b in range(x.shape[0]):
        maskf = (mask[b] != 0).astype(np.float32)
        nz = np.array([maskf[o:o + sz].sum() for o, sz in zip(MOD_OFF, SIZES)],
                      np.float64)
        m = dict(shared)
        m["x"] = np.ascontiguousarray(
            x[b].reshape(NT, 128, DIM).transpose(1, 0, 2))
        m["maskf"] = maskf
        m["mb2d"] = np.ascontiguousarray((MB * (1.0 - maskf)).reshape(8, 128).T)
        m["den3"] = np.where(nz > 0, 1.0 / np.maximum(nz * N, 1.0), 0.0).astype(np.float32)
        m["bias3"] = np.where(nz > 0, 0.0, MB).astype(np.float32)
        in_maps.append(m)
    return flags, in_maps


def kernel(**inputs):
    flags, in_maps = prepare(inputs)
    nc = get_program(flags)
    res = run_bass_kernel_spmd(nc, in_maps, list(range(len(in_maps))))
    out = np.stack([r["out0"].reshape(N, DIM) for r in res.results])
    return np.ascontiguousarray(out, dtype=np.float32)


# revision 27
# speedup vs baseline: 1.9997x; 1.9997x over previous
"""Trainium2 Bass kernel for nn_Block_69861938036758 (sparse_attention).

Data-parallel over batch B=8 across 8 NeuronCores (one batch element per
core, no collectives). Per core, one fused transformer block:

  LN1 -> per-modality QKV -> masked per-modality softmax with
  modality-importance weighting -> AV -> proj -> residual -> LN2 ->
  fc1 -> exact gelu -> fc2 -> residual

Layout choices (matmul operands bf16, fp32 accumulation everywhere):
  - Scores computed transposed: S^T[ktok, qtok], so the softmax key-sums are
    matmul contractions and the prob matrix is never transposed.
  - Per-modality row-sums ride free as an extra ones-column in V.
  - Modality-importance means via the factorization
      mean[m,h] = SCALE/(nz_m*N) * <qsum_h, ksum_{m,h}>.
  - The modality weight (softmaxed importance) is folded into the PSUM->SBUF
    evacuation of U as a per-partition tensor_scalar multiply; per-query
    normalization is then a single bf16 tensor_tensor DIVIDE against the
    broadcast row-sums (no reciprocal, no fp32 combine chain).
  - Weight DMAs ordered x -> wq -> wk -> wv; pw/f1/f2 prefetched on the sync
    queue during attention so the MLP never waits on HBM.
"""
import numpy as np
import ml_dtypes
from contextlib import ExitStack

import concourse.bass as bass
import concourse.bacc as bacc
import concourse.tile as tile
from concourse import mybir
from concourse.bass_utils import run_bass_kernel_spmd
from concourse.masks import make_identity

F32 = mybir.dt.float32
BF16 = mybir.dt.bfloat16
FP8 = mybir.dt.float8e4
DR = mybir.MatmulPerfMode.DoubleRow
AF = mybir.ActivationFunctionType
ALU = mybir.AluOpType

SIZES = (256, 512, 256)
H, DIM, HD, N = 12, 768, 64, 1024
NT = N // 128            # 8 token tiles
SCALE = HD ** -0.5
MB = -30000.0            # additive mask bias (exp underflows to exact 0)
MOD_OFF = (0, 256, 768)
MOD_CHUNKS = ((0, 2), (2, 6), (6, 8))   # ktok 128-chunk ranges per modality
MT_MOD = (0, 0, 1, 1, 1, 1, 2, 2)       # modality of each 128-token tile
EPS = 1e-5

_CACHE = {}


def _emit(ctx, tc, nc, T, flags, sfx="", out_name="out"):
    apply_ln1, apply_ln2, use_pb, use_f1b, use_f2b, need_eps = flags
    v = nc.vector
    s = nc.scalar
    pe = nc.tensor

    const = ctx.enter_context(tc.tile_pool(name="const" + sfx, bufs=1))
    drs = ctx.enter_context(tc.tile_pool(name="drs" + sfx, bufs=4, space="DRAM"))

    # x streams first so LN1 can start immediately.
    p_x = tc.alloc_tile_pool(name="p_x" + sfx, bufs=1)          # [A..E]
    x_sb = p_x.tile([128, NT, DIM], F32)
    nc.sync.dma_start(out=x_sb[:], in_=T["x"].ap())

    ident = const.tile([128, 128], BF16)
    make_identity(nc, ident[:])

    # const-table DMAs ride the gpsimd queue so the sync queue streams
    # x -> wq -> wk -> wv back-to-back.
    mb_sb = const.tile([128, 8], F32)
    nc.gpsimd.dma_start(out=mb_sb[:], in_=T["mb2d"].ap())
    maskrep = const.tile([128, N], F32)
    mf = T["maskf"].ap()
    nc.gpsimd.dma_start(out=maskrep[:], in_=bass.AP(
        tensor=mf.tensor, offset=mf.offset, ap=[[0, 128], [1, N]]))
    denr = const.tile([128, 3], F32)
    d3 = T["den3"].ap()
    nc.gpsimd.dma_start(out=denr[:], in_=bass.AP(
        tensor=d3.tensor, offset=d3.offset, ap=[[0, 128], [1, 3]]))
    br12 = const.tile([12, 3], F32)
    b3 = T["bias3"].ap()
    nc.gpsimd.dma_start(out=br12[:], in_=bass.AP(
        tensor=b3.tensor, offset=b3.offset, ap=[[0, 12], [1, 3]]))
    seg_sb = const.tile([128, 6, 12], BF16)
    nc.gpsimd.dma_start(out=seg_sb[:], in_=T["seg"].ap())
    eps_t = const.tile([128, 1], F32)
    v.memset(eps_t[:], EPS)
    # mscale[p, h, m]: modality weight for partitions 0..63, 1.0 at 64..127
    # (so the ones-column row-sum row of U rides through evacuation unscaled).
    mscale = const.tile([128, 12, 3], F32)
    v.memset(mscale[:], 1.0)

    def brep(name):
        t = const.tile([128, DIM], F32, tag=name)
        ap = T[name].ap()
        nc.sync.dma_start(out=t[:], in_=bass.AP(
            tensor=ap.tensor, offset=ap.offset, ap=[[0, 128], [1, DIM]]))
        return t

    ln1w_r = brep("ln1w") if apply_ln1 else None
    ln1b_r = brep("ln1b") if apply_ln1 else None
    ln2w_r = brep("ln2w") if apply_ln2 else None
    ln2b_r = brep("ln2b") if apply_ln2 else None
    pb_r = brep("pb") if use_pb else None
    f2b_r = brep("f2b") if use_f2b else None
    f1b_sb = None
    if use_f1b:
        f1b_sb = const.tile([128, 24], F32)
        nc.sync.dma_start(out=f1b_sb[:], in_=T["f1b"].ap())

    # manually-managed pools; stack (LIFO) allocator => nest lifetimes,
    # with weight-prefetch pools on the right-side stack.
    p_ot = tc.alloc_tile_pool(name="p_ot" + sfx, bufs=1)        # [A..E]
    p_qktv = tc.alloc_tile_pool(name="p_qktv" + sfx, bufs=1)    # [A..D]
    OT = p_ot.tile([128, 6, N], BF16)
    QT = p_qktv.tile([128, 6, N], BF16)
    # KTz: K^T stored twice with the other head's 64 rows zeroed, so score
    # matmuls are full-128-row (the PE HAM activity monitor ignores 64-row
    # matmuls and leaves the clock gate at half rate for all of phase D).
    KTz = p_qktv.tile([128, 2, 6, N], BF16)
    V = p_qktv.tile([128, NT, H, 80], FP8)   # 80 = 16B-aligned h-stride

    v.memset(V[:, :, :, HD:HD + 1], 1.0)  # ones column -> free row-sums
    v.memset(KTz[64:128, 0, :, :], 0.0)
    v.memset(KTz[0:64, 1, :, :], 0.0)

    # modality-importance statistics accumulate during phase B; released
    # after phase C (stacked above p_qktv, LIFO).
    stat = tc.alloc_tile_pool(name="stat" + sfx, bufs=1)
    statw = tc.alloc_tile_pool(name="statw" + sfx, bufs=2)
    qs = stat.tile([128, 6], F32)
    ks = stat.tile([128, 6, 3], F32)
    prod = stat.tile([128, 6, 3], BF16)

    def layer_norm_into(pool, src_ap, wr, br_, tag):
        """LN over free dim (768) -> bf16 [128, 768] tile."""
        stats = pool.tile([128, 3, 6], F32, tag=tag + "_st")
        for i in range(3):
            v.bn_stats(out=stats[:, i, :], in_=src_ap[:, i * 256:(i + 1) * 256])
        mv = pool.tile([128, 2], F32, tag=tag + "_mv")
        v.bn_aggr(out=mv[:], in_=stats[:])
        sd = pool.tile([128, 1], F32, tag=tag + "_sd")
        s.activation(sd[:], mv[:, 1:2], AF.Sqrt, bias=eps_t[:])
        rstd = pool.tile([128, 1], F32, tag=tag + "_rs")
        v.reciprocal(rstd[:], sd[:])
        out_bf = pool.tile([128, DIM], BF16, tag=tag + "_o")
        if wr is None:
            v.tensor_scalar(out_bf[:], src_ap, mv[:, 0:1], rstd[:],
                            op0=ALU.subtract, op1=ALU.mult)
        else:
            tmp = pool.tile([128, DIM], F32, tag=tag + "_t")
            v.tensor_scalar(tmp[:], src_ap, mv[:, 0:1], rstd[:],
                            op0=ALU.subtract, op1=ALU.mult)
            v.tensor_tensor(tmp[:], tmp[:], wr[:], op=ALU.mult)
            v.tensor_tensor(out_bf[:], tmp[:], br_[:], op=ALU.add)
        return out_bf

    # ---- Phase A: LN1 + transpose to xnT; Phase B: QKV ----------------------
    with tc.tile_pool(name="qkvw" + sfx, bufs=1) as qkvw:

        wq_sb = qkvw.tile([128, 3, 6, 6, 128], FP8)
        wk_sb = qkvw.tile([128, 3, 6, 6, 128], FP8)
        wv_sb = qkvw.tile([128, 3, 6, DIM], FP8)
        for m in range(3):
            nc.sync.dma_start(out=wq_sb[:, m, :, :, :], in_=T["wq"].ap()[:, m])
        for m in range(3):
            nc.sync.dma_start(out=wk_sb[:, m, :, :, :], in_=T["wk"].ap()[:, m])
        for m in range(3):
            nc.sync.dma_start(out=wv_sb[:, m, :, :], in_=T["wv"].ap()[:, m])
        xnT = qkvw.tile([128, 6, N], FP8)

        with tc.tile_pool(name="lnp" + sfx, bufs=3) as lnp, \
             tc.tile_pool(name="ptp" + sfx, bufs=3, space="PSUM") as ptp:
            for mt in range(NT):
                xn = layer_norm_into(lnp, x_sb[:, mt, :], ln1w_r, ln1b_r, "ln1")
                for kc in range(6):
                    pt = ptp.tile([128, 128], BF16, tag="tp")
                    pe.transpose(pt[:], xn[:, kc * 128:(kc + 1) * 128], ident[:])
                    s.copy(xnT[:, kc, mt * 128:(mt + 1) * 128], pt[:])

        with tc.tile_pool(name="pqv" + sfx, bufs=2, space="PSUM") as pqv:
            for w_sb, is_q in ((wq_sb, True), (wk_sb, False)):
                for pc in range(6):
                    ps = pqv.tile([128, N], F32, tag="q")
                    for seg, m in ((0, 0), (1, 1), (2, 1), (3, 2)):
                        o = seg * 256
                        for kc in range(0, 6, 2):
                            pe.matmul(ps[:, o:o + 256],
                                      w_sb[:, m, kc:kc + 2, pc, :],
                                      xnT[:, kc:kc + 2, o:o + 256],
                                      start=(kc == 0), stop=(kc == 4),
                                      perf_mode=DR)
                    if is_q:
                        s.copy(QT[:, pc, :], ps[:])
                        # importance stats ride along on otherwise-idle DVE
                        v.reduce_sum(qs[:, pc:pc + 1], QT[:, pc, :],
                                     axis=mybir.AxisListType.X)
                    else:
                        s.copy(KTz[0:64, 0, pc, :], ps[0:64, :])
                        s.copy(KTz[64:128, 1, pc, :], ps[64:128, :])
                        km = statw.tile([128, N], F32, tag="km")
                        v.tensor_tensor(km[0:64, :], KTz[0:64, 0, pc, :],
                                        maskrep[0:64, :], op=ALU.mult)
                        v.tensor_tensor(km[64:128, :], KTz[64:128, 1, pc, :],
                                        maskrep[64:128, :], op=ALU.mult)
                        for m in range(3):
                            o, sz = MOD_OFF[m], SIZES[m]
                            v.reduce_sum(ks[:, pc, m:m + 1], km[:, o:o + sz],
                                         axis=mybir.AxisListType.X)
            for mt in range(NT):
                m = MT_MOD[mt]
                ps = pqv.tile([128, DIM], F32, tag="v")
                for fo, fs in ((0, 512), (512, 256)):
                    for kc in range(0, 6, 2):
                        pe.matmul(ps[:, fo:fo + fs],
                                  xnT[:, kc:kc + 2, mt * 128:(mt + 1) * 128],
                                  wv_sb[:, m, kc:kc + 2, fo:fo + fs],
                                  start=(kc == 0), stop=(kc == 4),
                                  perf_mode=DR)
                # V evacuation on DVE keeps ACT clear for the first exps
                v.tensor_copy(V[:, mt, :, 0:HD],
                              ps[:].rearrange("p (h d) -> p h d", h=H))
                # (fp8 V: DoubleRow AV packs chunk pairs; pad cols 65..79 unused)
                if mt == 3:
                    # ---- Phase C: modality-importance means -> mscale ------
                    # Spliced mid-V-loop: the DVE stats are ready by now, the
                    # pm matmul slots between V matmuls with no PE stall, and
                    # the scm->mscale DMA round-trip (~3us SWDGE completion
                    # latency) finishes under the rest of the V-loop instead
                    # of stalling the first exp/evacuation of phase D.
                    for m_ in range(3):
                        v.tensor_scalar(ks[:, :, m_], ks[:, :, m_],
                                        denr[:, m_:m_ + 1], SCALE,
                                        op0=ALU.mult, op1=ALU.mult)
                    for pc in range(6):
                        v.tensor_scalar(prod[:, pc, :], ks[:, pc, :],
                                        qs[:, pc:pc + 1], None, op0=ALU.mult)
                    pm = pqv.tile([12, 3], F32, tag="v")  # borrow a v-slot
                    for pc in range(6):
                        pe.matmul(pm[:], seg_sb[:, pc, :], prod[:, pc, :],
                                  start=(pc == 0), stop=(pc == 5))
                    mn = stat.tile([12, 3], F32)
                    v.tensor_tensor(mn[:], pm[:], br12[:], op=ALU.add)
                    me = stat.tile([12, 3], F32)
                    msum = stat.tile([12, 1], F32)
                    s.activation(me[:], mn[:], AF.Exp, accum_out=msum[:])
                    mrec = stat.tile([12, 1], F32)
                    v.reciprocal(mrec[:], msum[:])
                    mult_sb = stat.tile([12, 3], F32)
                    v.tensor_scalar(mult_sb[:], me[:], mrec[:], None,
                                    op0=ALU.mult)
                    scm = drs.tile([12, 3], F32, tag="scm")
                    nc.gpsimd.dma_start(out=scm[:], in_=mult_sb[:])
                    sc = scm[:]
                    nc.gpsimd.dma_start(out=mscale[0:64, :, :], in_=bass.AP(
                        tensor=sc.tensor, offset=sc.offset,
                        ap=[[0, 64], [3, 12], [1, 3]]))

    statw.release()
    stat.release()

    # prefetch proj + MLP weights on the sync queue; they land during
    # attention so proj/fc1/fc2 never wait on HBM.
    p_pw = tc.alloc_tile_pool(name="p_pw" + sfx, bufs=1, side="right")
    pw_sb = p_pw.tile([128, 6, DIM], BF16)
    nc.sync.dma_start(out=pw_sb[:], in_=T["pw"].ap())
    p_w1 = tc.alloc_tile_pool(name="p_w1" + sfx, bufs=1, side="right")
    f1_sb = p_w1.tile([128, 6, 4 * DIM], BF16)
    nc.sync.dma_start(out=f1_sb[:], in_=T["f1"].ap())

    # ---- Phase D: attention (transposed scores) -----------------------------
    # Head-pair structure: the two heads of a pair share KT/QT partition
    # halves, so their score matmuls are emitted adjacently with row-group
    # tile positions (0,0)/(64,0) and run CONCURRENTLY in the PE array.
    # AV uses fp8 E/V with DoubleRow over chunk pairs (2 contraction tiles
    # per pass).  Row-sums collect as [16,64] blocks per (head, modality) so
    # the per-pair reciprocal is only 64 elements deep.
    with tc.tile_pool(name="pst" + sfx, bufs=2, space="PSUM") as pst, \
         tc.tile_pool(name="pu" + sfx, bufs=2, space="PSUM") as pu, \
         tc.tile_pool(name="ep" + sfx, bufs=4) as ep, \
         tc.tile_pool(name="usb" + sfx, bufs=9) as usb, \
         tc.tile_pool(name="r6p" + sfx, bufs=2) as r6p, \
         tc.tile_pool(name="rp" + sfx, bufs=9) as rp, \
         tc.tile_pool(name="cp" + sfx, bufs=3) as cp, \
         nc.allow_low_precision(reason="bf16/fp8 attention; tolerance 2e-2"):
        for hp in range(H // 2):
            hA, hB = 2 * hp, 2 * hp + 1
            pc = hp
            r6 = r6p.tile([96, 64], BF16, tag="r6", name=f"r6_{hp}")
            Us = {}
            Up = {}
            for m in range(3):
                c0, c1 = MOD_CHUNKS[m]
                Up[hA] = pu.tile([HD + 1, N], F32, tag="u", name=f"u_{hA}_{m}")
                Up[hB] = pu.tile([HD + 1, N], F32, tag="u", name=f"u_{hB}_{m}")
                for sp in range((c1 - c0) // 2):
                    ca, cb = c0 + 2 * sp, c0 + 2 * sp + 1
                    E2 = {}
                    for h in (hA, hB):
                        E2[h] = ep.tile([128, 2, N], FP8, tag="e",
                                        name=f"e_{h}_{sp}")
                    for ci, c in ((0, ca), (1, cb)):
                        sts = {}
                        for h in (hA, hB):
                            st = pst.tile([128, N], F32, tag="st",
                                          name=f"st_{h}_{c}")
                            for half in range(2):
                                hs = slice(half * 512, (half + 1) * 512)
                                pe.matmul(st[:, hs],
                                          KTz[:, h % 2, pc,
                                              c * 128:(c + 1) * 128],
                                          QT[:, pc, hs],
                                          start=True, stop=True)
                            sts[h] = st
                        for h in (hA, hB):
                            s.activation(E2[h][:, ci, :], sts[h][:],
                                         AF.Exp, bias=mb_sb[:, c:c + 1],
                                         scale=SCALE)
                    for h in (hA, hB):
                        for half in range(2):
                            hs = slice(half * 512, (half + 1) * 512)
                            pe.matmul(Up[h][:, hs],
                                      V[:, ca:ca + 2, h, 0:HD + 1],
                                      E2[h][:, :, hs],
                                      start=(sp == 0),
                                      stop=(sp == (c1 - c0) // 2 - 1),
                                      perf_mode=DR)
                # m-group complete for both heads: evacuate with the modality
                # weight folded in (row 64, the row-sum, is scaled by 1.0) and
                # scatter the row-sum into the pair collect tile.
                for h in (hA, hB):
                    Usb = usb.tile([HD + 1, N], BF16, tag="usb",
                                   name=f"usb_{h}_{m}")
                    v.tensor_scalar(Usb[:], Up[h][:],
                                    mscale[0:HD + 1, h, m:m + 1],
                                    None, op0=ALU.mult)
                    idx = (h % 2) * 3 + m
                    nc.sync.dma_start(out=r6[idx * 16:(idx + 1) * 16, :],
                                      in_=Usb[64:65, :])
                    Us[(h, m)] = Usb
            # pair complete: one shallow reciprocal serves both heads
            if need_eps:
                v.tensor_scalar(r6[:], r6[:], 1e-12, None, op0=ALU.add)
            v.reciprocal(r6[:], r6[:])
            rr = drs.tile([96, 64], BF16, tag="scr", name=f"rr_{hp}")
            nc.sync.dma_start(out=rr[:], in_=r6[:])
            for h in (hA, hB):
                po = (h % 2) * 64
                Rs = []
                for mm in range(3):
                    idx = (h % 2) * 3 + mm
                    ra = rr[:]
                    Rm = rp.tile([64, N], BF16, tag="rm", name=f"rm_{h}_{mm}")
                    nc.gpsimd.dma_start(out=Rm[:], in_=bass.AP(
                        tensor=ra.tensor, offset=ra.offset + idx * 16 * 64,
                        ap=[[0, 64], [64, 16], [1, 64]]))
                    Rs.append(Rm)
                acc = cp.tile([64, N], BF16, tag="acc", name=f"acc_{h}")
                v.tensor_tensor(acc[:], Us[(h, 0)][0:64, :], Rs[0][:],
                                op=ALU.mult)
                t1 = cp.tile([64, N], BF16, tag="t1")
                v.tensor_tensor(t1[:], Us[(h, 1)][0:64, :], Rs[1][:],
                                op=ALU.mult)
                v.tensor_tensor(acc[:], acc[:], t1[:], op=ALU.add)
                t2 = cp.tile([64, N], BF16, tag="t2")
                v.tensor_tensor(t2[:], Us[(h, 2)][0:64, :], Rs[2][:],
                                op=ALU.mult)
                v.tensor_tensor(OT[po:po + 64, pc, :], acc[:], t2[:],
                                op=ALU.add)

    p_qktv.release()

    # ---- Phase E+F fused: proj + residual + LN2 + transpose, per tile -------
    # One loop per token tile keeps PE (proj matmuls + transposes), DVE
    # (residual + LN2) and ACT (hT evacuation) pipelined instead of running
    # three serial engine-queues.
    p_x2h = tc.alloc_tile_pool(name="p_x2h" + sfx, bufs=1, side="right")  # [E..H]
    x2_sb = p_x2h.tile([128, NT, DIM], F32)
    hT = p_x2h.tile([128, 6, N], BF16)
    p_w2 = tc.alloc_tile_pool(name="p_w2" + sfx, bufs=1, side="right")
    f2_sb = p_w2.tile([128, 24, DIM], BF16)
    nc.sync.dma_start(out=f2_sb[:], in_=T["f2"].ap())
    with tc.tile_pool(name="py" + sfx, bufs=2, space="PSUM") as py, \
         tc.tile_pool(name="lnp2" + sfx, bufs=3) as lnp2, \
         tc.tile_pool(name="ptp2" + sfx, bufs=2, space="PSUM") as ptp2:
        for mt in range(NT):
            ps = py.tile([128, DIM], F32, tag="y")
            for fo, fs in ((0, 512), (512, 256)):
                for pc in range(6):
                    pe.matmul(ps[:, fo:fo + fs],
                              OT[:, pc, mt * 128:(mt + 1) * 128],
                              pw_sb[:, pc, fo:fo + fs],
                              start=(pc == 0), stop=(pc == 5))
            if use_pb:
                v.tensor_tensor(ps[:], ps[:], pb_r[:], op=ALU.add)
            v.tensor_tensor(x2_sb[:, mt, :], ps[:], x_sb[:, mt, :], op=ALU.add)
            hn = layer_norm_into(lnp2, x2_sb[:, mt, :], ln2w_r, ln2b_r, "ln2")
            for kc in range(6):
                pt = ptp2.tile([128, 128], BF16, tag="tp2")
                pe.transpose(pt[:], hn[:, kc * 128:(kc + 1) * 128], ident[:])
                s.copy(hT[:, kc, mt * 128:(mt + 1) * 128], pt[:])

    p_ot.release()
    p_x.release()

    # ---- Phase G/H: MLP -----------------------------------------------------
    with tc.tile_pool(name="mlpg" + sfx, bufs=1) as mlpg, \
         tc.tile_pool(name="pg" + sfx, bufs=2, space="PSUM") as pg, \
         tc.tile_pool(name="pz" + sfx, bufs=2, space="PSUM") as pz, \
         tc.tile_pool(name="op" + sfx, bufs=3) as op:
        gT = mlpg.tile([128, 24, N], BF16)
        # half-token passes: fc1 on tokens 0..511 starts as soon as the first
        # four LN2 tiles are transposed, overlapping the rest of phase F.
        for half in range(2):
            hs = slice(half * 512, (half + 1) * 512)
            for oc in range(24):
                ps = pg.tile([128, 512], F32, tag="g")
                for kc in range(6):
                    pe.matmul(ps[:], f1_sb[:, kc, oc * 128:(oc + 1) * 128],
                              hT[:, kc, hs], start=(kc == 0), stop=(kc == 5))
                if use_f1b:
                    s.activation(gT[:, oc, hs], ps[:], AF.Gelu,
                                 bias=f1b_sb[:, oc:oc + 1])
                else:
                    s.activation(gT[:, oc, hs], ps[:], AF.Gelu)
        for t in range(NT):
            ps = pz.tile([128, DIM], F32, tag="z")
            for fo, fs in ((0, 512), (512, 256)):
                for oc in range(24):
                    pe.matmul(ps[:, fo:fo + fs],
                              gT[:, oc, t * 128:(t + 1) * 128],
                              f2_sb[:, oc, fo:fo + fs],
                              start=(oc == 0), stop=(oc == 23))
            ob = op.tile([128, DIM], F32, tag="ob")
            if use_f2b:
                v.tensor_tensor(ob[:], ps[:], f2b_r[:], op=ALU.add)
                v.tensor_tensor(ob[:], ob[:], x2_sb[:, t, :], op=ALU.add)
            else:
                v.tensor_tensor(ob[:], ps[:], x2_sb[:, t, :], op=ALU.add)
            nc.gpsimd.dma_start(out=T[out_name].ap()[t], in_=ob[:])
    p_w2.release()
    p_x2h.release()
    p_w1.release()
    p_pw.release()


def _build(flags, reps=1):
    nc = bacc.Bacc("TRN2", target_bir_lowering=False, debug=False, num_devices=8)
    apply_ln1, apply_ln2, use_pb, use_f1b, use_f2b, need_eps = flags
    T = {}
    T["x"] = nc.dram_tensor("x", (128, NT, DIM), F32, kind="ExternalInput")
    T["maskf"] = nc.dram_tensor("maskf", (N,), F32, kind="ExternalInput")
    T["mb2d"] = nc.dram_tensor("mb2d", (128, 8), F32, kind="ExternalInput")
    T["den3"] = nc.dram_tensor("den3", (3,), F32, kind="ExternalInput")
    T["bias3"] = nc.dram_tensor("bias3", (3,), F32, kind="ExternalInput")
    T["seg"] = nc.dram_tensor("seg", (128, 6, 12), BF16, kind="ExternalInput")
    T["wq"] = nc.dram_tensor("wq", (128, 3, 6, 6, 128), FP8, kind="ExternalInput")
    T["wk"] = nc.dram_tensor("wk", (128, 3, 6, 6, 128), FP8, kind="ExternalInput")
    T["wv"] = nc.dram_tensor("wv", (128, 3, 6, DIM), FP8, kind="ExternalInput")
    T["pw"] = nc.dram_tensor("pw", (128, 6, DIM), BF16, kind="ExternalInput")
    T["f1"] = nc.dram_tensor("f1", (128, 6, 4 * DIM), BF16, kind="ExternalInput")
    T["f2"] = nc.dram_tensor("f2", (128, 24, DIM), BF16, kind="ExternalInput")
    if apply_ln1:
        T["ln1w"] = nc.dram_tensor("ln1w", (DIM,), F32, kind="ExternalInput")
        T["ln1b"] = nc.dram_tensor("ln1b", (DIM,), F32, kind="ExternalInput")
    if apply_ln2:
        T["ln2w"] = nc.dram_tensor("ln2w", (DIM,), F32, kind="ExternalInput")
        T["ln2b"] = nc.dram_tensor("ln2b", (DIM,), F32, kind="ExternalInput")
    if use_pb:
        T["pb"] = nc.dram_tensor("pb", (DIM,), F32, kind="ExternalInput")
    if use_f1b:
        T["f1b"] = nc.dram_tensor("f1b", (128, 24), F32, kind="ExternalInput")
    if use_f2b:
        T["f2b"] = nc.dram_tensor("f2b", (DIM,), F32, kind="ExternalInput")
    for r in range(reps):
        T[f"out{r}"] = nc.dram_tensor(f"out{r}", (NT, 128, DIM), F32,
                                      kind="ExternalOutput")

    with tile.TileContext(nc) as tc:
        for r in range(reps):
            with ExitStack() as ctx:
                _emit(ctx, tc, nc, T, flags, sfx=f"_{r}", out_name=f"out{r}")
    nc.compile()
    return nc


def get_program(flags, reps=1):
    key = (flags, reps)
    if key not in _CACHE:
        _CACHE[key] = _build(flags, reps)
    return _CACHE[key]


def _bf(a):
    return np.ascontiguousarray(a, dtype=np.float32).astype(ml_dtypes.bfloat16)


def _f8(a):
    a = np.clip(np.ascontiguousarray(a, dtype=np.float32), -240.0, 240.0)
    return a.astype(ml_dtypes.float8_e4m3)


def prepare(inputs):
    """Host-side prep: flags + per-core input maps."""
    x = np.asarray(inputs["x"], np.float32)
    mask = np.asarray(inputs["attention_mask"])
    ln1_w = np.asarray(inputs["ln1_w"], np.float32)
    ln1_b = np.asarray(inputs["ln1_b"], np.float32)
    ln2_w = np.asarray(inputs["ln2_w"], np.float32)
    ln2_b = np.asarray(inputs["ln2_b"], np.float32)
    proj_b = np.asarray(inputs["proj_b"], np.float32)
    fc1_b = np.asarray(inputs["fc1_b"], np.float32)
    fc2_b = np.asarray(inputs["fc2_b"], np.float32)
    qkv_ws = [np.asarray(inputs[k], np.float32)
              for k in ("qkv_text_w", "qkv_video_w", "qkv_audio_w")]
    proj_w = np.asarray(inputs["proj_w"], np.float32)
    fc1_w = np.asarray(inputs["fc1_w"], np.float32)
    fc2_w = np.asarray(inputs["fc2_w"], np.float32)

    nz_all = []
    for b in range(x.shape[0]):
        mf = (mask[b] != 0)
        nz_all.append([mf[o:o + sz].sum() for o, sz in zip(MOD_OFF, SIZES)])
    flags = (
        not (np.all(ln1_w == 1.0) and np.all(ln1_b == 0.0)),
        not (np.all(ln2_w == 1.0) and np.all(ln2_b == 0.0)),
        bool(np.any(proj_b != 0.0)),
        bool(np.any(fc1_b != 0.0)),
        bool(np.any(fc2_b != 0.0)),
        bool(np.any(np.array(nz_all) == 0)),
    )
    apply_ln1, apply_ln2, use_pb, use_f1b, use_f2b, need_eps = flags

    # shared (identical per core) tensors, DMA-friendly partition-first layouts
    def pack_qk(rows):
        # (3 modalities, 768 out, 768 in) -> [128 p(featin), 3, 6 kc, 6 pc, 128]
        a = np.stack([w[rows[0]:rows[1]].T.reshape(6, 128, 6, 128)
                      for w in qkv_ws])            # (3, kc, p, pc, j)
        return _f8(a.transpose(2, 0, 1, 3, 4))

    wq = pack_qk((0, 768))
    wk = pack_qk((768, 1536))
    wv = _f8(np.stack([w[1536:2304].T.reshape(6, 128, DIM) for w in qkv_ws])
             .transpose(2, 0, 1, 3))               # [128, 3, 6, 768]
    pw = _bf(proj_w.T.reshape(6, 128, DIM).transpose(1, 0, 2))
    f1 = _bf(fc1_w.T.reshape(6, 128, 4 * DIM).transpose(1, 0, 2))
    f2 = _bf(fc2_w.T.reshape(24, 128, DIM).transpose(1, 0, 2))
    seg = np.zeros((128, 6, 12), np.float32)
    for pc in range(6):
        seg[0:64, pc, 2 * pc] = 1.0
        seg[64:128, pc, 2 * pc + 1] = 1.0
    shared = {"wq": wq, "wk": wk, "wv": wv, "pw": pw, "f1": f1, "f2": f2,
              "seg": _bf(seg)}
    if apply_ln1:
        shared["ln1w"], shared["ln1b"] = ln1_w, ln1_b
    if apply_ln2:
        shared["ln2w"], shared["ln2b"] = ln2_w, ln2_b
    if use_pb:
        shared["pb"] = proj_b
    if use_f1b:
        shared["f1b"] = np.ascontiguousarray(fc1_b.reshape(24, 128).T)
    if use_f2b:
        shared["f2b"] = fc2_b

    in_maps = []
    for b in range(x.shape[0]):
        maskf = (mask[b] != 0).astype(np.float32)
        nz = np.array([maskf[o:o + sz].sum() for o, sz in zip(MOD_OFF, SIZES)],
                      np.float64)
        m = dict(shared)
        m["x"] = np.ascontiguousarray(
            x[b].reshape(NT, 128, DIM).transpose(1, 0, 2))
        m["maskf"] = maskf
        m["mb2d"] = np.ascontiguousarray((MB * (1.0 - maskf)).reshape(8, 128).T)
        m["den3"] = np.where(nz > 0, 1.0 / np.maximum(nz * N, 1.0), 0.0).astype(np.float32)
        m["bias3"] = np.where(nz > 0, 0.0, MB).astype(np.float32)
        in_maps.append(m)
    return flags, in_maps


def kernel(**inputs):
    flags, in_maps = prepare(inputs)
    nc = get_program(flags)
    res = run_bass_kernel_spmd(nc, in_maps, list(range(len(in_maps))))
    out = np.stack([r["out0"].reshape(N, DIM) for r in res.results])
    return np.ascontiguousarray(out, dtype=np.float32)
